# revision 1
# baseline (speedup 1.0000x reference)
"""Distributed quantum-circuit state-vector kernel for 8 Trainium2 NeuronCores.

Problem: state (2, 2^23) f32 (real/imag channels), 4 gates of 128x128
"complex" matmuls (Karatsuba form with a channel swap per gate).

Algebraic reduction (verified vs the reference to ~6.5e-7 rel err):
  Writing z = s[0] + i*s[1] and each gate as z' = i*conj(z @ Ug^T) applied on a
  fixed 7-qubit axis, gates 0..2 all act on the low 7 bits and gate 3 on bits
  9..15 of the flat amplitude index.  Composing all four gates:
      out = conj(U3) @ Z @ B   per (q0..8)-indexed 128x128 block,
      B = U0^T @ conj(U1)^T @ U2^T,  out ch0 = Re, ch1 = Im.
  Sharding the 512 leading blocks 64-per-core is then embarrassingly parallel.

Per-block dataflow on each core (PE computes lhsT.T @ rhs):
  psumY = Z0^T @ [CrT|CiT] + Z1^T @ [-CiT|CrT]     (= [YrT | YiT], partition=b)
  yt    = copy(psumY)                               (DVE, PSUM->SBUF)
  psumO = Yr @ [Br|Bi] + Yi @ [-Bi|Br]              (= [zr | zi], partition=k)
  outS  = copy(psumO)                               (ACT, PSUM->SBUF)
with Cr = U3r, Ci = -U3i (C = conj(U3)).

Measured on HW (loop-slope method, 8 cores, final 16-rep run): 52.1 us per
pass overall, 44.8 us steady-state, vs the 16.78 MiB/core / 358 GB/s = 46.9 us
memory roofline -- memory-bound, compute fully hidden (PE 256 fp32r MMs
~34 us, DVE 23 us, ACT 23 us; a compute-free DMA echo of the same traffic
measures the same pass time).  Key measured
facts: fp32r MM N=256 ~133 ns (1 cyc/row incl self weight-load; plain fp32 is
4 cyc/row); PSUM->SBUF copies are overhead-bound below 512 elems/partition
(DVE 864 ns @256 vs 721 ns @512), hence the block-pair structure; in-DMAs on
the SP HWDGE queue and out-DMAs on the GPSIMD SWDGE queue (single-queue issue
cost ~10 us/pass); host-side shard transposes make every DMA contiguous per
partition.  Rel err vs reference: 2.07e-4 (fp32r input rounding).
"""

import numpy as np

import concourse.bass as bass
import concourse.bacc as bacc
import concourse.mybir as mybir
import concourse.tile as tile
from concourse.bass_utils import run_bass_kernel_spmd

N_CORES = 8
N_QUBITS = 23
BLOCKS = 512              # 2^9 leading (q0..q8) blocks of 128x128 amplitudes
BPC = BLOCKS // N_CORES   # 64 blocks per core
CH = 16                   # blocks per DMA chunk (1 MiB per channel per chunk)
NCHUNK = BPC // CH
F32 = mybir.dt.float32
# PE matmul dtype: float32r streams rows at full rate for N>=256 (plain
# float32 is 4 cycles/row).  All matmul operands must be produced as MM_DT
# end-to-end (BIR verifier requires fp32r inputs to be rounded to fp32r).
MM_DT = mybir.dt.float32r

_cached_nc = {}


def _build(passes=1, loop=0, mode="full"):
    """Build the per-core Bass program.

    passes > 1 (python-unrolled) or loop > 0 (hardware For_i) repeats the
    whole computation, writing all but the final pass to internal DRAM
    scratch -- used only for slope-based HW timing (the container has no
    NTFF profiling hook).  mode strips stages from the LOOPED passes only
    (A/B bottleneck isolation): full | nostep2 | nodve | noout | noin."""
    key = (passes, loop, mode)
    if key in _cached_nc:
        return _cached_nc[key]

    nc = bacc.Bacc(
        "TRN2", target_bir_lowering=False, debug=False, num_devices=N_CORES
    )
    # [c, a, g, b]: host pre-transposes so every in-DMA is contiguous per
    # partition (a = q9..15 of the block, g = block index, b = q16..22)
    state_d = nc.dram_tensor(
        "state_sh", (2, 128, BPC, 128), MM_DT, kind="ExternalInput"
    ).ap()
    cc1_d = nc.dram_tensor("cc1", (128, 256), MM_DT, kind="ExternalInput").ap()
    cc2_d = nc.dram_tensor("cc2", (128, 256), MM_DT, kind="ExternalInput").ap()
    bb1_d = nc.dram_tensor("bb1", (128, 256), MM_DT, kind="ExternalInput").ap()
    bb2_d = nc.dram_tensor("bb2", (128, 256), MM_DT, kind="ExternalInput").ap()
    # [c, k, g, k2]: host transposes back after the run
    out_d = nc.dram_tensor(
        "out_sh", (2, 128, BPC, 128), F32, kind="ExternalOutput"
    ).ap()
    n_scratch = min(2, passes - 1) + (1 if loop else 0)
    scratch = [
        nc.dram_tensor(f"scratch{i}", (2, 128, BPC, 128), F32).ap()
        for i in range(n_scratch)
    ]

    with tile.TileContext(nc) as tc:
        with (
            tc.tile_pool(name="const", bufs=1) as cpool,
            tc.tile_pool(name="io", bufs=3) as iop,
            tc.tile_pool(name="mid", bufs=6) as midp,
            tc.tile_pool(name="ps", bufs=4, space=bass.MemorySpace.PSUM) as psp,
        ):
            cc1 = cpool.tile([128, 256], MM_DT, tag="cc1")
            cc2 = cpool.tile([128, 256], MM_DT, tag="cc2")
            bb1 = cpool.tile([128, 256], MM_DT, tag="bb1")
            bb2 = cpool.tile([128, 256], MM_DT, tag="bb2")
            nc.sync.dma_start(cc1[:], cc1_d[:])
            nc.sync.dma_start(cc2[:], cc2_d[:])
            nc.sync.dma_start(bb1[:], bb1_d[:])
            nc.sync.dma_start(bb2[:], bb2_d[:])

            if loop:
                with tc.For_i(0, loop, 1, hint_engines=(mybir.EngineType.PE,)):
                    for c in range(NCHUNK):
                        _emit_chunk(
                            nc, iop, midp, psp, state_d, scratch[-1],
                            cc1, cc2, bb1, bb2, c, mode=mode,
                        )
            for p in range(passes):
                dst = out_d if p == passes - 1 else scratch[p % 2]
                for c in range(NCHUNK):
                    _emit_chunk(nc, iop, midp, psp, state_d, dst, cc1, cc2, bb1, bb2, c)

    nc.compile()
    _cached_nc[key] = nc
    return nc


def _emit_chunk(nc, iop, midp, psp, state_d, out_d, cc1, cc2, bb1, bb2, c,
                mode="full"):
    """Blocks are processed in PAIRS so each PSUM stage fills a whole 2 KiB
    bank (512 f32) and each PSUM->SBUF copy moves 512 elems/partition --
    PSUM-read copies are overhead-dominated below that (measured 864 ns for
    256 wide vs 721 ns for 512 wide on DVE)."""
    H = CH // 2
    in0 = iop.tile([128, CH, 128], MM_DT, tag="in0")
    in1 = iop.tile([128, CH, 128], MM_DT, tag="in1")
    outS = iop.tile([128, CH, 2, 128], F32, tag="outS")
    # dram [g, a, b] -> sbuf [a, g, b]; half-chunk granularity so compute can
    # start after the first half lands and stores drain before the chunk ends.
    if mode != "noin":
        # first chunk: quarter-granularity loads so the first matmuls start
        # after ~256 KB instead of ~512 KB (shaves the pipeline-fill latency)
        nsplit = 4 if c == 0 else 2
        Q = CH // nsplit
        for h in range(nsplit):
            hs = slice(c * CH + h * Q, c * CH + (h + 1) * Q)
            ts = slice(h * Q, (h + 1) * Q)
            nc.sync.dma_start(in0[:, ts], state_d[0, :, hs, :])
            nc.sync.dma_start(in1[:, ts], state_d[1, :, hs, :])
    for j in range(0, CH, 2):
        psY = psp.tile([128, 512], F32, tag="psY")
        for s in range(2):
            sl2 = slice(s * 256, (s + 1) * 256)
            nc.tensor.matmul(psY[:, sl2], in0[:, j + s], cc1[:], start=True, stop=False)
            nc.tensor.matmul(psY[:, sl2], in1[:, j + s], cc2[:], start=False, stop=True)
        yt = midp.tile([128, 512], MM_DT, tag="yt")
        if mode == "actdve":
            nc.scalar.copy(yt[:], psY[:])
        else:
            nc.vector.tensor_copy(yt[:], psY[:])
        outap = outS[:, j : j + 2].rearrange("p g c k -> p (g c k)")
        if mode == "nostep2":
            nc.scalar.copy(outap, yt[:].bitcast(F32))
        else:
            psO = psp.tile([128, 512], F32, tag="psO")
            for s in range(2):
                sl2 = slice(s * 256, (s + 1) * 256)
                nc.tensor.matmul(
                    psO[:, sl2], yt[:, s * 256 : s * 256 + 128], bb1[:],
                    start=True, stop=False,
                )
                nc.tensor.matmul(
                    psO[:, sl2], yt[:, s * 256 + 128 : s * 256 + 256], bb2[:],
                    start=False, stop=True,
                )
            if mode == "dveact":
                nc.vector.tensor_copy(outap, psO[:])
            else:
                nc.scalar.copy(outap, psO[:])
        if (j + 2) % H == 0 and mode != "noout":
            h = j // H
            hs = slice(c * CH + h * H, c * CH + (h + 1) * H)
            ts = slice(h * H, (h + 1) * H)
            # sbuf [k, g, c, k2] -> dram [c, k, g, k2]; gpsimd SWDGE queue so
            # the SP sequencer only dispatches the input DMAs
            nc.gpsimd.dma_start(out_d[0, :, hs, :], outS[:, ts, 0, :])
            nc.gpsimd.dma_start(out_d[1, :, hs, :], outS[:, ts, 1, :])


def _host_matrices(U):
    """Compose the fixed gate matrices on the host (float64, then f32)."""
    U64 = np.asarray(U, dtype=np.float64)
    Uc = U64[:, 0] + 1j * U64[:, 1]
    B = Uc[0].T @ np.conj(Uc[1]).T @ Uc[2].T
    C = np.conj(Uc[3])
    Br = B.real.astype(np.float32)
    Bi = B.imag.astype(np.float32)
    Cr = C.real.astype(np.float32)
    Ci = C.imag.astype(np.float32)
    cc1 = np.ascontiguousarray(np.concatenate([Cr.T, Ci.T], axis=1))
    cc2 = np.ascontiguousarray(np.concatenate([-Ci.T, Cr.T], axis=1))
    bb1 = np.ascontiguousarray(np.concatenate([Br, Bi], axis=1))
    bb2 = np.ascontiguousarray(np.concatenate([-Bi, Br], axis=1))
    return cc1, cc2, bb1, bb2


def _shard_state(state):
    """(2, 2^23) -> per-core [c, a, g, b] shards."""
    S = np.asarray(state, dtype=np.float32).reshape(2, BLOCKS, 128, 128)
    return [
        np.ascontiguousarray(
            S[:, k * BPC : (k + 1) * BPC].transpose(0, 2, 1, 3)
        )
        for k in range(N_CORES)
    ]


def _gather_out(outs):
    """per-core [c, k, g, k2] -> (2, 2^23)."""
    full = np.concatenate([o.transpose(0, 2, 1, 3) for o in outs], axis=1)
    return np.ascontiguousarray(full).reshape(2, 2**N_QUBITS)


def run(state, U, **spmd_kwargs):
    U = np.asarray(U, dtype=np.float32)
    cc1, cc2, bb1, bb2 = _host_matrices(U)
    shards = _shard_state(state)
    nc = _build()
    in_maps = [
        {
            "state_sh": shards[k],
            "cc1": cc1,
            "cc2": cc2,
            "bb1": bb1,
            "bb2": bb2,
        }
        for k in range(N_CORES)
    ]
    res = run_bass_kernel_spmd(
        nc, in_maps, core_ids=list(range(N_CORES)), **spmd_kwargs
    )
    return _gather_out([res.results[k]["out_sh"] for k in range(N_CORES)]), res


def kernel(state, U):
    out, _ = run(state, U)
    return out


# ---------------------------------------------------------------------------
# Benchmarking: no NTFF profiling hook exists in this container, so HW time is
# measured as the wall-clock slope between an R-pass NEFF and the 1-pass NEFF
# with device-resident inputs (cancels RPC/dispatch/launch overhead).
# ---------------------------------------------------------------------------


def _make_exec(nc):
    import jax
    from concourse.bass2jax import (
        _bass_exec_p,
        install_neuronx_cc_hook,
        partition_id_tensor,
    )
    from jax.experimental.shard_map import shard_map
    from jax.sharding import Mesh, NamedSharding, PartitionSpec

    install_neuronx_cc_hook()
    partition_name = (
        nc.partition_id_tensor.name if nc.partition_id_tensor else None
    )
    in_names, out_names, out_avals, zero_outs = [], [], [], []
    for alloc in nc.m.functions[0].allocations:
        if not isinstance(alloc, mybir.MemoryLocationSet):
            continue
        name = alloc.memorylocations[0].name
        if alloc.kind == "ExternalInput":
            if name != partition_name:
                in_names.append(name)
        elif alloc.kind == "ExternalOutput":
            out_names.append(name)
            shape = tuple(alloc.tensor_shape)
            dtype = mybir.dt.np(alloc.dtype)
            out_avals.append(jax.core.ShapedArray(shape, dtype))
            zero_outs.append(np.zeros(shape, dtype))
    n_params = len(in_names)
    all_in = in_names + out_names
    if partition_name is not None:
        all_in = all_in + [partition_name]

    def _body(*args):
        operands = list(args)
        if partition_name is not None:
            operands.append(partition_id_tensor())
        outs = _bass_exec_p.bind(
            *operands,
            out_avals=tuple(out_avals),
            in_names=tuple(all_in),
            out_names=tuple(out_names),
            lowering_input_output_aliases=(),
            sim_require_finite=True,
            sim_require_nnan=True,
            nc=nc,
        )
        return tuple(outs)

    devices = jax.devices()[:N_CORES]
    mesh = Mesh(np.asarray(devices), ("core",))
    spec = PartitionSpec("core")
    nin = n_params + len(out_names)
    fn = jax.jit(
        shard_map(
            _body,
            mesh=mesh,
            in_specs=(spec,) * nin,
            out_specs=(spec,) * len(out_names),
            check_rep=False,
        ),
        keep_unused=True,
    )
    sharding = NamedSharding(mesh, spec)
    return fn, in_names[:n_params], zero_outs, sharding


def _time_nc(nc, feeds, reps=8):
    """Compile nc, run with device-resident inputs, return list of wall times.

    feeds: name -> np.ndarray (broadcast to all cores) or list of per-core
    arrays."""
    import time

    import jax

    fn, names, zero_outs, sharding = _make_exec(nc)

    def put(v):
        vs = v if isinstance(v, list) else [v] * N_CORES
        return jax.device_put(np.concatenate(vs, axis=0), sharding)

    args = [put(feeds[n]) for n in names]
    args += [put(z) for z in zero_outs]
    jax.block_until_ready(fn(*args))  # compile + warmup
    times = []
    for _ in range(reps):
        t0 = time.perf_counter()
        jax.block_until_ready(fn(*args))
        times.append(time.perf_counter() - t0)
    return times


def _state_feeds(state, U):
    cc1, cc2, bb1, bb2 = _host_matrices(np.asarray(U, dtype=np.float32))
    return {
        "state_sh": _shard_state(state),
        "cc1": cc1,
        "cc2": cc2,
        "bb1": bb1,
        "bb2": bb2,
    }


def _slope(builder, feeds, loops, reps=8, per=1):
    """Interleave timing rounds across loop counts to cancel drift."""
    import time

    import jax

    runners = {}
    for nloop in loops:
        nc = builder(nloop)
        fn, names, zero_outs, sharding = _make_exec(nc)

        def put(v):
            vs = v if isinstance(v, list) else [v] * N_CORES
            return jax.device_put(np.concatenate(vs, axis=0), sharding)

        args = [put(feeds[n]) for n in names] + [put(z) for z in zero_outs]
        jax.block_until_ready(fn(*args))  # compile + warmup
        runners[nloop] = (fn, args)

    results = {nloop: [] for nloop in loops}
    for _ in range(reps):
        for nloop in loops:
            fn, args = runners[nloop]
            t0 = time.perf_counter()
            jax.block_until_ready(fn(*args))
            results[nloop].append(time.perf_counter() - t0)
    for nloop in loops:
        times = results[nloop]
        print(
            f"loop={nloop}: min={min(times)*1e6:.1f}us "
            f"median={sorted(times)[len(times)//2]*1e6:.1f}us"
        )
    xs = np.array(sorted(results))
    ys = np.array([min(results[p]) for p in xs])
    slope = np.polyfit(xs, ys, 1)[0] if len(xs) > 1 else float("nan")
    return slope * 1e9 / per, results


def bench(state, U, loops=(64, 512, 1024), reps=8):
    feeds = _state_feeds(state, U)
    return _slope(lambda nl: _build(passes=1, loop=nl), feeds, loops, reps)


# --- micro benchmarks ------------------------------------------------------

_micro_cache = {}


def _build_micro(kind, loop, mm_per_iter=8):
    """kind: 'mm' = back-to-back fp32r matmuls; 'dma' = pure 16MiB echo."""
    key = (kind, loop, mm_per_iter)
    if key in _micro_cache:
        return _micro_cache[key]
    nc = bacc.Bacc(
        "TRN2", target_bir_lowering=False, debug=False, num_devices=N_CORES
    )
    if kind in ("mm", "mmw"):
        a_d = nc.dram_tensor("a", (128, 256), MM_DT, kind="ExternalInput").ap()
        out_d = nc.dram_tensor("o", (128, 256), F32, kind="ExternalOutput").ap()
        with tile.TileContext(nc) as tc:
            with (
                tc.tile_pool(name="sb", bufs=1) as sb,
                tc.tile_pool(name="ps", bufs=1, space=bass.MemorySpace.PSUM) as psp,
            ):
                a = sb.tile([128, 256], MM_DT, tag="a")
                nc.sync.dma_start(a[:], a_d[:])
                # mmw: 8 distinct stationary tiles, rotating -> measures the
                # per-MM self-weight-load cost when weights change every MM
                ws = sb.tile([128, 8, 128], MM_DT, tag="ws")
                for w in range(8):
                    nc.vector.tensor_copy(ws[:, w], a[:, 0:128])
                with tc.For_i(0, loop, 1):
                    for i in range(mm_per_iter):
                        ps = psp.tile([128, 256], F32, tag=f"ps{i % 6}")
                        lhsT = a[:, 0:128] if kind == "mm" else ws[:, i % 8]
                        nc.tensor.matmul(
                            ps[:], lhsT, a[:], start=True, stop=True
                        )
                ps2 = psp.tile([128, 256], F32, tag="pso")
                nc.tensor.matmul(ps2[:], a[:, 0:128], a[:], start=True, stop=True)
                o = sb.tile([128, 256], F32, tag="o")
                nc.vector.tensor_copy(o[:], ps2[:])
                nc.sync.dma_start(out_d[:], o[:])
    elif kind in ("dvecopy", "actcopy", "dvecopy512", "actcopy512",
                  "dvecopy128", "sbcopy", "dvetrans"):
        W = 512 if "512" in kind else (128 if "128" in kind else 256)
        a_d = nc.dram_tensor("a", (128, 256), MM_DT, kind="ExternalInput").ap()
        out_d = nc.dram_tensor("o", (128, 256), F32, kind="ExternalOutput").ap()
        with tile.TileContext(nc) as tc:
            with (
                tc.tile_pool(name="sb", bufs=1) as sb,
                tc.tile_pool(name="ps", bufs=1, space=bass.MemorySpace.PSUM) as psp,
            ):
                a = sb.tile([128, 256], MM_DT, tag="a")
                nc.sync.dma_start(a[:], a_d[:])
                ps = psp.tile([128, W], F32, tag="ps")
                for w in range(0, W, 256):
                    nc.tensor.matmul(
                        ps[:, w : w + 256], a[:, 0:128], a[:], start=True, stop=True
                    )
                src_sb = sb.tile([128, W], F32, tag="src")
                nc.vector.tensor_copy(src_sb[:], ps[:])
                with tc.For_i(0, loop, 1):
                    for i in range(mm_per_iter):
                        t = sb.tile([128, W], F32, tag=f"t{i % 8}")
                        if kind == "sbcopy":
                            nc.vector.tensor_copy(t[:], src_sb[:])
                        elif kind == "dvetrans":
                            nc.vector.transpose(t[:, 0:128], src_sb[:, 0:128])
                        elif kind.startswith("dvecopy"):
                            nc.vector.tensor_copy(t[:], ps[:])
                        else:
                            nc.scalar.copy(t[:], ps[:])
                o = sb.tile([128, 256], F32, tag="o")
                nc.vector.tensor_copy(o[:], ps[:, 0:256])
                nc.sync.dma_start(out_d[:], o[:])
    elif kind == "dma":
        state_d = nc.dram_tensor(
            "state_sh", (2, 128, BPC, 128), MM_DT, kind="ExternalInput"
        ).ap()
        out_d = nc.dram_tensor(
            "out_sh", (2, 128, BPC, 128), F32, kind="ExternalOutput"
        ).ap()
        with tile.TileContext(nc) as tc:
            with tc.tile_pool(name="io", bufs=3) as iop:
                with tc.For_i(0, loop, 1):
                    for c in range(NCHUNK):
                        sl = slice(c * CH, (c + 1) * CH)
                        in0 = iop.tile([128, CH, 128], MM_DT, tag="in0")
                        in1 = iop.tile([128, CH, 128], MM_DT, tag="in1")
                        nc.sync.dma_start(in0[:], state_d[0, :, sl, :])
                        nc.sync.dma_start(in1[:], state_d[1, :, sl, :])
                        nc.gpsimd.dma_start(out_d[0, :, sl, :], in0[:].bitcast(F32))
                        nc.gpsimd.dma_start(out_d[1, :, sl, :], in1[:].bitcast(F32))
    nc.compile()
    _micro_cache[key] = nc
    return nc


def bench_micro(kind, state=None, U=None, loops=(64, 512, 1024), reps=8, mm_per_iter=8):
    if kind.startswith(("mm", "mmw", "dvecopy", "actcopy", "sbcopy", "dvetrans")):
        feeds = {"a": np.random.randn(128, 256).astype(np.float32)}
        per = mm_per_iter
    else:
        sf = _state_feeds(state, U)
        feeds = {"state_sh": sf["state_sh"]}
        per = 1
    return _slope(
        lambda nl: _build_micro(kind, nl, mm_per_iter), feeds, loops, reps, per=per
    )



# revision 3
# speedup vs baseline: 1.1586x; 1.1586x over previous
"""Distributed quantum-circuit state-vector kernel for 8 Trainium2 NeuronCores.

Problem: state (2, 2^23) f32 (real/imag channels), 4 gates of 128x128
"complex" matmuls (Karatsuba form with a channel swap per gate).

Algebraic reduction (verified vs the reference to ~6.5e-7 rel err in f32):
  Writing z = s[0] + i*s[1] and each gate as z' = i*conj(z @ Ug^T) applied on a
  fixed 7-qubit axis, gates 0..2 all act on the low 7 bits and gate 3 on bits
  9..15 of the flat amplitude index.  Composing all four gates:
      out = conj(U3) @ Z @ B   per (q0..8)-indexed 128x128 block,
      B = U0^T @ conj(U1)^T @ U2^T,  out ch0 = Re, ch1 = Im.
  Sharding the 512 leading blocks 64-per-core is then embarrassingly parallel.

Precision/bandwidth choice: the whole pipeline runs in bfloat16 (state shards,
gate matrices, intermediate Y, and the output written back to DRAM; the host
up-casts to f32).  That halves HBM traffic vs f32 -- 8.39 MiB/core/pass at the
~358 GB/s per-core share = ~23.4 us memory roofline -- and costs ~2.4e-3 rel
err (vs the 2e-2 gate).  PE streams bf16 at 1 row/cycle, so the 256 matmuls
(64 blocks x 2 stages x 2 accumulating MMs of N=256) are the critical path:
~(262 cyc / 2.4 GHz) * 256 = ~28 us warm.

Per-block dataflow on each core (PE computes lhsT.T @ rhs):
  psumY = Z0^T @ [CrT|CiT] + Z1^T @ [-CiT|CrT]     (= [YrT | YiT], partition=b)
  yt    = copy(psumY)                               (DVE, PSUM->SBUF, ->bf16)
  psumO = Yr @ [Br|Bi] + Yi @ [-Bi|Br]              (= [zr | zi], partition=k)
  outS  = copy(psumO)                               (ACT, PSUM->SBUF, ->bf16)
with Cr = U3r, Ci = -U3i (C = conj(U3)).

Engine budgets per pass (cost-model): DMA 23.4 us aggregate, PE ~28 us,
DVE 32 x (120+512)/0.96GHz = 21 us, ACT 32 x (172+512)/1.2GHz = 18 us.
In-DMAs ride the SP HWDGE queue, out-DMAs the GPSIMD SWDGE queue; host-side
shard transposes keep every DMA contiguous (2 KiB runs per partition).
"""

import numpy as np

import concourse.bass as bass
import concourse.bacc as bacc
import concourse.mybir as mybir
import concourse.tile as tile
from concourse.bass_utils import run_bass_kernel_spmd

N_CORES = 8
N_QUBITS = 23
BLOCKS = 512              # 2^9 leading (q0..q8) blocks of 128x128 amplitudes
BPC = BLOCKS // N_CORES   # 64 blocks per core
CH = 16                   # blocks per DMA chunk (512 KiB per channel per chunk)
NCHUNK = BPC // CH
F32 = mybir.dt.float32
# bf16 end-to-end: PE streams 1 row/cycle, DMA traffic halves vs f32, and the
# quantization error (~2.4e-3 rel) is far inside the 2e-2 gate.
MM_DT = mybir.dt.bfloat16
NP_BF16 = mybir.dt.np(MM_DT)

_cached_nc = {}


def _build(passes=1, loop=0, mode="full"):
    """Build the per-core Bass program.

    passes > 1 (python-unrolled) or loop > 0 (hardware For_i) repeats the
    whole computation, writing all but the final pass to internal DRAM
    scratch -- used only for slope-based HW timing (the container has no
    NTFF profiling hook).  mode strips stages from the LOOPED passes only
    (A/B bottleneck isolation): full | nostep2 | nodve | noout | noin."""
    key = (passes, loop, mode)
    if key in _cached_nc:
        return _cached_nc[key]

    nc = bacc.Bacc(
        "TRN2", target_bir_lowering=False, debug=False, num_devices=N_CORES
    )
    # [c, a, g, b]: host pre-transposes so every in-DMA is contiguous per
    # partition (a = q9..15 of the block, g = block index, b = q16..22)
    state_d = nc.dram_tensor(
        "state_sh", (2, 128, BPC, 128), MM_DT, kind="ExternalInput"
    ).ap()
    cc1_d = nc.dram_tensor("cc1", (128, 256), MM_DT, kind="ExternalInput").ap()
    cc2_d = nc.dram_tensor("cc2", (128, 256), MM_DT, kind="ExternalInput").ap()
    bb1_d = nc.dram_tensor("bb1", (128, 256), MM_DT, kind="ExternalInput").ap()
    bb2_d = nc.dram_tensor("bb2", (128, 256), MM_DT, kind="ExternalInput").ap()
    # [c, k, g, k2]: host transposes back (and up-casts) after the run
    out_d = nc.dram_tensor(
        "out_sh", (2, 128, BPC, 128), MM_DT, kind="ExternalOutput"
    ).ap()
    n_scratch = min(2, passes - 1) + (1 if loop else 0)
    scratch = [
        nc.dram_tensor(f"scratch{i}", (2, 128, BPC, 128), MM_DT).ap()
        for i in range(n_scratch)
    ]

    with tile.TileContext(nc) as tc:
        with (
            tc.tile_pool(name="const", bufs=1) as cpool,
            tc.tile_pool(name="io", bufs=3) as iop,
            tc.tile_pool(name="mid", bufs=6) as midp,
            tc.tile_pool(name="ps", bufs=4, space=bass.MemorySpace.PSUM) as psp,
        ):
            cc1 = cpool.tile([128, 256], MM_DT, tag="cc1")
            cc2 = cpool.tile([128, 256], MM_DT, tag="cc2")
            bb1 = cpool.tile([128, 256], MM_DT, tag="bb1")
            bb2 = cpool.tile([128, 256], MM_DT, tag="bb2")
            nc.sync.dma_start(cc1[:], cc1_d[:])
            nc.sync.dma_start(cc2[:], cc2_d[:])
            nc.sync.dma_start(bb1[:], bb1_d[:])
            nc.sync.dma_start(bb2[:], bb2_d[:])

            if loop:
                with tc.For_i(0, loop, 1, hint_engines=(mybir.EngineType.PE,)):
                    for c in range(NCHUNK):
                        _emit_chunk(
                            nc, iop, midp, psp, state_d, scratch[-1],
                            cc1, cc2, bb1, bb2, c, mode=mode,
                        )
            for p in range(passes):
                dst = out_d if p == passes - 1 else scratch[p % 2]
                for c in range(NCHUNK):
                    _emit_chunk(nc, iop, midp, psp, state_d, dst, cc1, cc2, bb1, bb2, c)

    nc.compile()
    _cached_nc[key] = nc
    return nc


def _emit_chunk(nc, iop, midp, psp, state_d, out_d, cc1, cc2, bb1, bb2, c,
                mode="full"):
    """Blocks are processed in PAIRS so each PSUM stage fills a whole 2 KiB
    bank (512 f32) and each PSUM->SBUF copy moves 512 elems/partition --
    PSUM-read copies are overhead-dominated below that."""
    H = CH // 2
    in0 = iop.tile([128, CH, 128], MM_DT, tag="in0")
    in1 = iop.tile([128, CH, 128], MM_DT, tag="in1")
    outS = iop.tile([128, CH, 2, 128], MM_DT, tag="outS")
    # dram [g, a, b] -> sbuf [a, g, b]; half-chunk granularity so compute can
    # start after the first half lands and stores drain before the chunk ends.
    if mode != "noin":
        # first chunk: quarter-granularity loads so the first matmuls start
        # after ~128 KB instead of ~256 KB (shaves the pipeline-fill latency)
        nsplit = 4 if c == 0 else 2
        Q = CH // nsplit
        for h in range(nsplit):
            hs = slice(c * CH + h * Q, c * CH + (h + 1) * Q)
            ts = slice(h * Q, (h + 1) * Q)
            nc.sync.dma_start(in0[:, ts], state_d[0, :, hs, :])
            nc.sync.dma_start(in1[:, ts], state_d[1, :, hs, :])
    for j in range(0, CH, 2):
        psY = psp.tile([128, 512], F32, tag="psY")
        for s in range(2):
            sl2 = slice(s * 256, (s + 1) * 256)
            nc.tensor.matmul(psY[:, sl2], in0[:, j + s], cc1[:], start=True, stop=False)
            nc.tensor.matmul(psY[:, sl2], in1[:, j + s], cc2[:], start=False, stop=True)
        yt = midp.tile([128, 512], MM_DT, tag="yt")
        if mode == "actdve":
            nc.scalar.copy(yt[:], psY[:])
        else:
            nc.vector.tensor_copy(yt[:], psY[:])
        outap = outS[:, j : j + 2].rearrange("p g c k -> p (g c k)")
        if mode == "nostep2":
            nc.scalar.copy(outap, yt[:])
        else:
            psO = psp.tile([128, 512], F32, tag="psO")
            for s in range(2):
                sl2 = slice(s * 256, (s + 1) * 256)
                nc.tensor.matmul(
                    psO[:, sl2], yt[:, s * 256 : s * 256 + 128], bb1[:],
                    start=True, stop=False,
                )
                nc.tensor.matmul(
                    psO[:, sl2], yt[:, s * 256 + 128 : s * 256 + 256], bb2[:],
                    start=False, stop=True,
                )
            if mode == "dveact":
                nc.vector.tensor_copy(outap, psO[:])
            else:
                nc.scalar.copy(outap, psO[:])
        if (j + 2) % H == 0 and mode != "noout":
            h = j // H
            hs = slice(c * CH + h * H, c * CH + (h + 1) * H)
            ts = slice(h * H, (h + 1) * H)
            # sbuf [k, g, c, k2] -> dram [c, k, g, k2]; gpsimd SWDGE queue so
            # the SP sequencer only dispatches the input DMAs
            nc.gpsimd.dma_start(out_d[0, :, hs, :], outS[:, ts, 0, :])
            nc.gpsimd.dma_start(out_d[1, :, hs, :], outS[:, ts, 1, :])


def _host_matrices(U):
    """Compose the fixed gate matrices on the host (float64, then bf16)."""
    U64 = np.asarray(U, dtype=np.float64)
    Uc = U64[:, 0] + 1j * U64[:, 1]
    B = Uc[0].T @ np.conj(Uc[1]).T @ Uc[2].T
    C = np.conj(Uc[3])
    Br = B.real.astype(NP_BF16)
    Bi = B.imag.astype(NP_BF16)
    Cr = C.real.astype(NP_BF16)
    Ci = C.imag.astype(NP_BF16)
    cc1 = np.ascontiguousarray(np.concatenate([Cr.T, Ci.T], axis=1))
    cc2 = np.ascontiguousarray(np.concatenate([-Ci.T, Cr.T], axis=1))
    bb1 = np.ascontiguousarray(np.concatenate([Br, Bi], axis=1))
    bb2 = np.ascontiguousarray(np.concatenate([-Bi, Br], axis=1))
    return cc1, cc2, bb1, bb2


def _shard_state(state):
    """(2, 2^23) f32 -> per-core bf16 [c, a, g, b] shards."""
    S = np.asarray(state, dtype=np.float32).astype(NP_BF16)
    S = S.reshape(2, BLOCKS, 128, 128)
    return [
        np.ascontiguousarray(
            S[:, k * BPC : (k + 1) * BPC].transpose(0, 2, 1, 3)
        )
        for k in range(N_CORES)
    ]


def _gather_out(outs):
    """per-core bf16 [c, k, g, k2] -> (2, 2^23) f32."""
    full = np.concatenate(
        [np.asarray(o).astype(np.float32).transpose(0, 2, 1, 3) for o in outs],
        axis=1,
    )
    return np.ascontiguousarray(full).reshape(2, 2**N_QUBITS)


def run(state, U, **spmd_kwargs):
    U = np.asarray(U, dtype=np.float32)
    cc1, cc2, bb1, bb2 = _host_matrices(U)
    shards = _shard_state(state)
    nc = _build()
    in_maps = [
        {
            "state_sh": shards[k],
            "cc1": cc1,
            "cc2": cc2,
            "bb1": bb1,
            "bb2": bb2,
        }
        for k in range(N_CORES)
    ]
    res = run_bass_kernel_spmd(
        nc, in_maps, core_ids=list(range(N_CORES)), **spmd_kwargs
    )
    return _gather_out([res.results[k]["out_sh"] for k in range(N_CORES)]), res


def kernel(state, U):
    out, _ = run(state, U)
    return out


# ---------------------------------------------------------------------------
# Benchmarking: no NTFF profiling hook exists in this container, so HW time is
# measured as the wall-clock slope between an R-pass NEFF and the 1-pass NEFF
# with device-resident inputs (cancels RPC/dispatch/launch overhead).
# ---------------------------------------------------------------------------


def _make_exec(nc):
    import jax
    from concourse.bass2jax import (
        _bass_exec_p,
        install_neuronx_cc_hook,
        partition_id_tensor,
    )
    from jax.experimental.shard_map import shard_map
    from jax.sharding import Mesh, NamedSharding, PartitionSpec

    install_neuronx_cc_hook()
    partition_name = (
        nc.partition_id_tensor.name if nc.partition_id_tensor else None
    )
    in_names, out_names, out_avals, zero_outs = [], [], [], []
    for alloc in nc.m.functions[0].allocations:
        if not isinstance(alloc, mybir.MemoryLocationSet):
            continue
        name = alloc.memorylocations[0].name
        if alloc.kind == "ExternalInput":
            if name != partition_name:
                in_names.append(name)
        elif alloc.kind == "ExternalOutput":
            out_names.append(name)
            shape = tuple(alloc.tensor_shape)
            dtype = mybir.dt.np(alloc.dtype)
            out_avals.append(jax.core.ShapedArray(shape, dtype))
            zero_outs.append(np.zeros(shape, dtype))
    n_params = len(in_names)
    all_in = in_names + out_names
    if partition_name is not None:
        all_in = all_in + [partition_name]

    def _body(*args):
        operands = list(args)
        if partition_name is not None:
            operands.append(partition_id_tensor())
        outs = _bass_exec_p.bind(
            *operands,
            out_avals=tuple(out_avals),
            in_names=tuple(all_in),
            out_names=tuple(out_names),
            lowering_input_output_aliases=(),
            sim_require_finite=True,
            sim_require_nnan=True,
            nc=nc,
        )
        return tuple(outs)

    devices = jax.devices()[:N_CORES]
    mesh = Mesh(np.asarray(devices), ("core",))
    spec = PartitionSpec("core")
    nin = n_params + len(out_names)
    fn = jax.jit(
        shard_map(
            _body,
            mesh=mesh,
            in_specs=(spec,) * nin,
            out_specs=(spec,) * len(out_names),
            check_rep=False,
        ),
        keep_unused=True,
    )
    sharding = NamedSharding(mesh, spec)
    return fn, in_names[:n_params], zero_outs, sharding


def _time_nc(nc, feeds, reps=8):
    """Compile nc, run with device-resident inputs, return list of wall times.

    feeds: name -> np.ndarray (broadcast to all cores) or list of per-core
    arrays."""
    import time

    import jax

    fn, names, zero_outs, sharding = _make_exec(nc)

    def put(v):
        vs = v if isinstance(v, list) else [v] * N_CORES
        return jax.device_put(np.concatenate(vs, axis=0), sharding)

    args = [put(feeds[n]) for n in names]
    args += [put(z) for z in zero_outs]
    jax.block_until_ready(fn(*args))  # compile + warmup
    times = []
    for _ in range(reps):
        t0 = time.perf_counter()
        jax.block_until_ready(fn(*args))
        times.append(time.perf_counter() - t0)
    return times


def _state_feeds(state, U):
    cc1, cc2, bb1, bb2 = _host_matrices(np.asarray(U, dtype=np.float32))
    return {
        "state_sh": _shard_state(state),
        "cc1": cc1,
        "cc2": cc2,
        "bb1": bb1,
        "bb2": bb2,
    }


def _slope(builder, feeds, loops, reps=8, per=1):
    """Interleave timing rounds across loop counts to cancel drift."""
    import time

    import jax

    runners = {}
    for nloop in loops:
        nc = builder(nloop)
        fn, names, zero_outs, sharding = _make_exec(nc)

        def put(v):
            vs = v if isinstance(v, list) else [v] * N_CORES
            return jax.device_put(np.concatenate(vs, axis=0), sharding)

        args = [put(feeds[n]) for n in names] + [put(z) for z in zero_outs]
        jax.block_until_ready(fn(*args))  # compile + warmup
        runners[nloop] = (fn, args)

    results = {nloop: [] for nloop in loops}
    for _ in range(reps):
        for nloop in loops:
            fn, args = runners[nloop]
            t0 = time.perf_counter()
            jax.block_until_ready(fn(*args))
            results[nloop].append(time.perf_counter() - t0)
    for nloop in loops:
        times = results[nloop]
        print(
            f"loop={nloop}: min={min(times)*1e6:.1f}us "
            f"median={sorted(times)[len(times)//2]*1e6:.1f}us"
        )
    xs = np.array(sorted(results))
    ys = np.array([min(results[p]) for p in xs])
    slope = np.polyfit(xs, ys, 1)[0] if len(xs) > 1 else float("nan")
    return slope * 1e9 / per, results


def bench(state, U, loops=(64, 512, 1024), reps=8):
    feeds = _state_feeds(state, U)
    return _slope(lambda nl: _build(passes=1, loop=nl), feeds, loops, reps)


# --- micro benchmarks ------------------------------------------------------

_micro_cache = {}


def _build_micro(kind, loop, mm_per_iter=8):
    """kind: 'mm' = back-to-back bf16 matmuls; 'dma' = pure 8MiB echo."""
    key = (kind, loop, mm_per_iter)
    if key in _micro_cache:
        return _micro_cache[key]
    nc = bacc.Bacc(
        "TRN2", target_bir_lowering=False, debug=False, num_devices=N_CORES
    )
    if kind in ("mm", "mmw"):
        a_d = nc.dram_tensor("a", (128, 256), MM_DT, kind="ExternalInput").ap()
        out_d = nc.dram_tensor("o", (128, 256), F32, kind="ExternalOutput").ap()
        with tile.TileContext(nc) as tc:
            with (
                tc.tile_pool(name="sb", bufs=1) as sb,
                tc.tile_pool(name="ps", bufs=1, space=bass.MemorySpace.PSUM) as psp,
            ):
                a = sb.tile([128, 256], MM_DT, tag="a")
                nc.sync.dma_start(a[:], a_d[:])
                # mmw: 8 distinct stationary tiles, rotating -> measures the
                # per-MM self-weight-load cost when weights change every MM
                ws = sb.tile([128, 8, 128], MM_DT, tag="ws")
                for w in range(8):
                    nc.vector.tensor_copy(ws[:, w], a[:, 0:128])
                with tc.For_i(0, loop, 1):
                    for i in range(mm_per_iter):
                        ps = psp.tile([128, 256], F32, tag=f"ps{i % 6}")
                        lhsT = a[:, 0:128] if kind == "mm" else ws[:, i % 8]
                        nc.tensor.matmul(
                            ps[:], lhsT, a[:], start=True, stop=True
                        )
                ps2 = psp.tile([128, 256], F32, tag="pso")
                nc.tensor.matmul(ps2[:], a[:, 0:128], a[:], start=True, stop=True)
                o = sb.tile([128, 256], F32, tag="o")
                nc.vector.tensor_copy(o[:], ps2[:])
                nc.sync.dma_start(out_d[:], o[:])
    elif kind in ("dvecopy", "actcopy", "dvecopy512", "actcopy512",
                  "dvecopy128", "sbcopy", "dvetrans"):
        W = 512 if "512" in kind else (128 if "128" in kind else 256)
        a_d = nc.dram_tensor("a", (128, 256), MM_DT, kind="ExternalInput").ap()
        out_d = nc.dram_tensor("o", (128, 256), F32, kind="ExternalOutput").ap()
        with tile.TileContext(nc) as tc:
            with (
                tc.tile_pool(name="sb", bufs=1) as sb,
                tc.tile_pool(name="ps", bufs=1, space=bass.MemorySpace.PSUM) as psp,
            ):
                a = sb.tile([128, 256], MM_DT, tag="a")
                nc.sync.dma_start(a[:], a_d[:])
                ps = psp.tile([128, W], F32, tag="ps")
                for w in range(0, W, 256):
                    nc.tensor.matmul(
                        ps[:, w : w + 256], a[:, 0:128], a[:], start=True, stop=True
                    )
                src_sb = sb.tile([128, W], F32, tag="src")
                nc.vector.tensor_copy(src_sb[:], ps[:])
                with tc.For_i(0, loop, 1):
                    for i in range(mm_per_iter):
                        t = sb.tile([128, W], MM_DT, tag=f"t{i % 8}")
                        if kind == "sbcopy":
                            nc.vector.tensor_copy(t[:], src_sb[:])
                        elif kind == "dvetrans":
                            nc.vector.transpose(t[:, 0:128], src_sb[:, 0:128])
                        elif kind.startswith("dvecopy"):
                            nc.vector.tensor_copy(t[:], ps[:])
                        else:
                            nc.scalar.copy(t[:], ps[:])
                o = sb.tile([128, 256], F32, tag="o")
                nc.vector.tensor_copy(o[:], ps[:, 0:256])
                nc.sync.dma_start(out_d[:], o[:])
    elif kind == "dma":
        state_d = nc.dram_tensor(
            "state_sh", (2, 128, BPC, 128), MM_DT, kind="ExternalInput"
        ).ap()
        out_d = nc.dram_tensor(
            "out_sh", (2, 128, BPC, 128), MM_DT, kind="ExternalOutput"
        ).ap()
        with tile.TileContext(nc) as tc:
            with tc.tile_pool(name="io", bufs=3) as iop:
                with tc.For_i(0, loop, 1):
                    for c in range(NCHUNK):
                        sl = slice(c * CH, (c + 1) * CH)
                        in0 = iop.tile([128, CH, 128], MM_DT, tag="in0")
                        in1 = iop.tile([128, CH, 128], MM_DT, tag="in1")
                        nc.sync.dma_start(in0[:], state_d[0, :, sl, :])
                        nc.sync.dma_start(in1[:], state_d[1, :, sl, :])
                        nc.gpsimd.dma_start(out_d[0, :, sl, :], in0[:])
                        nc.gpsimd.dma_start(out_d[1, :, sl, :], in1[:])
    nc.compile()
    _micro_cache[key] = nc
    return nc


def bench_micro(kind, state=None, U=None, loops=(64, 512, 1024), reps=8, mm_per_iter=8):
    if kind.startswith(("mm", "mmw", "dvecopy", "actcopy", "sbcopy", "dvetrans")):
        feeds = {"a": np.random.randn(128, 256).astype(NP_BF16)}
        per = mm_per_iter
    else:
        sf = _state_feeds(state, U)
        feeds = {"state_sh": sf["state_sh"]}
        per = 1
    return _slope(
        lambda nl: _build_micro(kind, nl, mm_per_iter), feeds, loops, reps, per=per
    )
